# revision 14
# baseline (speedup 1.0000x reference)
"""DeepLSTM Trainium2 kernel (nn_DeepLSTM_1365799600435).

Strategy: data-parallel over batch (B=128 -> 16 rows/core on 8 cores, no
collectives). Per core:
  Phase A: x-path 3-layer MLPs (4 gates) precomputed for all T in big
           weight-stationary bf16 matmuls; result xa spilled to DRAM.
           Wx3 is pre-scaled by 2^18 so xa arrives at the same scale as
           the fp8 h-path output (see below).
  Phase B: sequential LSTM recurrence over T=1024 steps. Weight-stationary
           orientation keeps every layer's activations in [feature, batch]
           form so no transposes are needed. h-path weights are fp8e4m3
           scaled by 64 per layer (halves LDWEIGHTS traffic, which binds
           the matmul issue rate at N=16); activations stay bf16 (mixed
           dtype matmuls). The cumulative 64^3 = 2^18 scale is divided
           out for free in the sigmoid/tanh activation `scale` param.
           Gates are computed in order Ch,F,I,O with per-gate relus and
           adds so each layer transition's PE->DVE semaphore latency
           hides under the other gates' matmuls. Dummy matmuls fill the
           cell-update tail to keep the PE HAM clock-gate at 2.4 GHz.
  Phase C: attention over T. exp without max-subtraction (logits are
           tanh-bounded), strided accumulation, fp32 accumulators.

All dynamic addressing uses register-offset APs on compute instructions
(this toolchain rejects register-offset DMA), with xa staged per
super-chunk through static DMAs.
"""

import os
import sys

import numpy as np
import ml_dtypes

for _p in ("/opt/trn_rl_repo", "/root/.axon_site/_ro/trn_rl_repo"):
    if os.path.isdir(_p) and _p not in sys.path:
        sys.path.append(_p)

import concourse.bass as bass
import concourse.mybir as mybir
import concourse.tile as tile
from concourse.bass import ds

F32 = mybir.dt.float32
BF16 = mybir.dt.bfloat16
F8 = mybir.dt.float8e4
WH_SCALE = 64.0  # h-weights stored as fp8e4m3 scaled by 64 (per layer)
DESCALE = 1.0 / (WH_SCALE ** 3)  # folded into the gate activation scale
AF = mybir.ActivationFunctionType

# Problem constants
B, T_FULL, IN, H = 128, 1024, 128, 256
M1 = M2 = 512
G = 4
NCORE = 8
BSH = B // NCORE  # 16 batch rows per core

CHUNK = 32          # recurrence steps per For_i iteration
CCOLS = CHUNK * BSH  # cols per chunk

N_DUMMY = int(os.environ.get("KERNEL_NDUMMY", "0"))
DUMMY_N = int(os.environ.get("KERNEL_DUMMYN", "384"))

_LDW_OPT = os.environ.get("KERNEL_LDW_OPT", "1") == "1"
_ldw_patched = [False]


def _patch_walrus_ldw_opt():
    if _ldw_patched[0] or not _LDW_OPT:
        return
    import concourse.bass_utils as _bu
    _orig = _bu.run_command

    def _patched(argv, **kw):
        argv = ["--enable-ldw-opt=true" if a == "--enable-ldw-opt=false" else a
                for a in argv]
        return _orig(argv, **kw)

    _bu.run_command = _patched
    _ldw_patched[0] = True


def _make_self_loading(nc):
    """Fold standalone InstLdweights into their matmuls (required by
    walrus --enable-ldw-opt=true, which overlaps weight loads with the
    previous matmul via the background weight buffer)."""
    n_conv = 0
    for func in nc.m.functions:
        for block in func.blocks:
            insts = block.instructions
            keep = []
            for inst in insts:
                cls = type(inst).__name__
                if cls == "InstLdweights":
                    n_conv += 1
                    if inst.sync_info and (inst.sync_info.on_wait or inst.sync_info.on_update):
                        nop = mybir.InstNoOp(name=nc.get_next_instruction_name(),
                                             engine=inst.engine, sync_info=inst.sync_info,
                                             bass_nofuse=True)
                        keep.append(nop)
                    continue
                if cls == "InstMatmult":
                    inst.ldweights = True
                keep.append(inst)
            insts[:] = keep
    return n_conv


def _legalize_waits(nc):
    """This walrus build accepts at most 1 sem-wait per instruction (2 for
    EventSemaphore ops). Tile sometimes attaches more (final drain, loop
    reset blocks): hoist extras onto same-engine NoOps inserted before."""
    n_split = 0
    for func in nc.m.functions:
        for block in func.blocks:
            insts = block.instructions
            i = 0
            while i < len(insts):
                inst = insts[i]
                si = inst.sync_info
                if si is None or not si.on_wait:
                    i += 1
                    continue
                cap = 2 if "EventSemaphore" in type(inst).__name__ else 1
                waits = list(si.on_wait)
                if len(waits) <= cap:
                    i += 1
                    continue
                keep, hoist = waits[-cap:], waits[:-cap]
                carriers = [
                    mybir.InstNoOp(
                        name=nc.get_next_instruction_name(),
                        engine=inst.engine,
                        sync_info=mybir.SyncInfo(on_wait=[w], on_update=[]),
                        bass_nofuse=True,
                    )
                    for w in hoist
                ]
                inst.sync_info = mybir.SyncInfo(on_wait=keep, on_update=list(si.on_update))
                insts[i:i] = carriers
                n_split += 1
                i += 1 + len(carriers)
    return n_split


# Gate order for phase B emission: Ch first so the t12/cnew/tanh chain can
# overlap the later gates' matmuls; O last so only add_O -> sigmoid -> h is
# exposed after the final matmul burst.
# Gate indices in the weight layout: F=0, I=1, O=2, Ch=3.
GATE_ORDER = (3, 0, 1, 2)


def build(T=T_FULL, sc_chunks=8, debug=False, phases="ABC"):
    """Build the per-core Bass program. T must be a multiple of 32."""
    assert T % 32 == 0
    COLS = T * BSH
    NCHUNK = COLS // CCOLS              # recurrence chunks
    sc_chunks = min(sc_chunks, NCHUNK)
    assert NCHUNK % sc_chunks == 0
    NSC = NCHUNK // sc_chunks           # super-chunks
    SCCOLS = sc_chunks * CCOLS          # cols per super-chunk
    NBLK = COLS // 512                  # 512-col blocks for phases A and C

    nc = bass.Bass()

    # ---- DRAM I/O (host pre-arranges layouts; see kernel()) ----
    xT_d = nc.dram_tensor("xT", [IN, COLS], BF16, kind="ExternalInput")
    wx1_d = nc.dram_tensor("wx1", [128, G * 512], BF16, kind="ExternalInput")
    wx2_d = nc.dram_tensor("wx2", [128, G * 4 * 512], BF16, kind="ExternalInput")
    wx3_d = nc.dram_tensor("wx3", [128, G * 4 * 256], BF16, kind="ExternalInput")
    wh1_d = nc.dram_tensor("wh1", [128, G * 2 * 512], F8, kind="ExternalInput")
    wh2_d = nc.dram_tensor("wh2", [128, G * 4 * 512], F8, kind="ExternalInput")
    wh3_d = nc.dram_tensor("wh3", [128, G * 4 * 256], F8, kind="ExternalInput")
    wa_d = nc.dram_tensor("wa", [128, 2 * 256], BF16, kind="ExternalInput")
    out_d = nc.dram_tensor("out", [2, 128, BSH], F32, kind="ExternalOutput")

    # xa spill: [gm, p, col]; gm = g*2 + j (j = output h-chunk), col = t*16+b
    xa_d = nc.dram_tensor("xa_d", [2 * G, 128, COLS], BF16,
                          kind="ExternalOutput" if debug else "Internal")
    hs_dump = nc.dram_tensor("hs_dump", [128, 2, COLS], BF16,
                             kind="ExternalOutput") if debug else None

    # ================= Phase A: x-path MLPs =================
    if "A" in phases:
      with tile.TileContext(nc) as tc:
          with (
              tc.tile_pool(name="a_w", bufs=1) as wpool,
              tc.tile_pool(name="a_ps", bufs=8, space="PSUM") as pspool,
              tc.tile_pool(name="a_sb", bufs=3) as spool,
          ):
              xT = wpool.tile([128, COLS], BF16)
              wx1 = wpool.tile([128, G * 512], BF16)
              wx2 = wpool.tile([128, G * 4 * 512], BF16)
              wx3 = wpool.tile([128, G * 4 * 256], BF16)
              nc.sync.dma_start(out=xT[:], in_=xT_d[:])
              nc.sync.dma_start(out=wx1[:], in_=wx1_d[:])
              nc.sync.dma_start(out=wx2[:], in_=wx2_d[:])
              nc.sync.dma_start(out=wx3[:], in_=wx3_d[:])

              for blk in range(NBLK):
                  c0 = blk * 512
                  for g in range(G):
                      # L1: [128 in] -> 512, K=1 chunk
                      ps1 = [pspool.tile([128, 512], F32, tag="ps", name=f"ps1_{blk}_{g}_{i}") for i in range(4)]
                      for mc in range(4):
                          nc.tensor.matmul(
                              out=ps1[mc][:],
                              lhsT=wx1[:, g * 512 + mc * 128 : g * 512 + (mc + 1) * 128],
                              rhs=xT[:, c0 : c0 + 512],
                              start=True, stop=True,
                          )
                      act1 = spool.tile([128, 4, 512], BF16, tag="act1")
                      for mc in range(4):
                          nc.vector.tensor_scalar_max(act1[:, mc, :], ps1[mc][:], 0.0)
                      # L2: 512 -> 512, K=4 chunks
                      ps2 = [pspool.tile([128, 512], F32, tag="ps", name=f"ps2_{blk}_{g}_{i}") for i in range(4)]
                      for mc in range(4):
                          for kc in range(4):
                              nc.tensor.matmul(
                                  out=ps2[mc][:],
                                  lhsT=wx2[:, (g * 4 + kc) * 512 + mc * 128 : (g * 4 + kc) * 512 + (mc + 1) * 128],
                                  rhs=act1[:, kc, :],
                                  start=(kc == 0), stop=(kc == 3),
                              )
                      act2 = spool.tile([128, 4, 512], BF16, tag="act2")
                      for mc in range(4):
                          nc.scalar.activation(act2[:, mc, :], ps2[mc][:], AF.Relu)
                      # L3: 512 -> 256, K=4 chunks
                      ps3 = [pspool.tile([128, 512], F32, tag="ps", name=f"ps3_{blk}_{g}_{i}") for i in range(2)]
                      for mc in range(2):
                          for kc in range(4):
                              nc.tensor.matmul(
                                  out=ps3[mc][:],
                                  lhsT=wx3[:, (g * 4 + kc) * 256 + mc * 128 : (g * 4 + kc) * 256 + (mc + 1) * 128],
                                  rhs=act2[:, kc, :],
                                  start=(kc == 0), stop=(kc == 3),
                              )
                      xa_sb = spool.tile([128, 2, 512], BF16, tag="xa_sb")
                      for mc in range(2):
                          nc.vector.tensor_copy(xa_sb[:, mc, :], ps3[mc][:])
                      nc.sync.dma_start(
                          out=xa_d[2 * g : 2 * g + 2, :, c0 : c0 + 512].rearrange("j p c -> p j c"),
                          in_=xa_sb[:],
                      )

    # ================= Phases B + C =================
    with tile.TileContext(nc) as tc:
        from contextlib import ExitStack
        with (
            tc.tile_pool(name="b_w", bufs=1) as wpool,
            tc.tile_pool(name="b_state", bufs=1) as stpool,
        ):
            bstack = ExitStack()
            pspool = bstack.enter_context(tc.tile_pool(name="b_ps", bufs=1, space="PSUM"))
            spool = bstack.enter_context(tc.tile_pool(name="b_sb", bufs=2))
            wh1 = wpool.tile([128, G * 2 * 512], F8)
            wh2 = wpool.tile([128, G * 4 * 512], F8)
            wh3 = wpool.tile([128, G * 4 * 256], F8)
            wa = wpool.tile([128, 2 * 256], BF16)
            nc.sync.dma_start(out=wh1[:], in_=wh1_d[:])
            nc.sync.dma_start(out=wh2[:], in_=wh2_d[:])
            nc.sync.dma_start(out=wh3[:], in_=wh3_d[:])
            nc.sync.dma_start(out=wa[:], in_=wa_d[:])

            # hs history dump (debug only; attention is folded into the loop)
            hsb = stpool.tile([128, 2, COLS], BF16) if debug else None
            cpspool = bstack.enter_context(tc.tile_pool(name="c_ps", bufs=2, space="PSUM"))
            cacc = stpool.tile([128, 2, 512], F32)
            nacc = stpool.tile([128, 2, 512], F32)
            nc.vector.memset(cacc[:], 0.0)
            nc.vector.memset(nacc[:], 0.0)
            # z: [c | tanh(Ch)] -- z[:, 0:32] is the persistent cell state
            z = stpool.tile([128, 64], F32)
            hstage = stpool.tile([128, 2, CCOLS], BF16)  # chunk history staging
            nc.vector.memset(z[:], 0.0)
            nc.vector.memset(hstage[:], 0.0)

            # xa staging: [p, gm, SCCOLS] per super-chunk (single buffer)
            xa_bufs = [stpool.tile([128, 2 * G, SCCOLS], BF16, name="xab0")]

            def load_sc(sc):
                buf = xa_bufs[sc % len(xa_bufs)]
                nc.sync.dma_start(
                    out=buf[:],
                    in_=xa_d[:, :, sc * SCCOLS : (sc + 1) * SCCOLS].rearrange("g p c -> p g c"),
                )
                return buf

            for sc in range(NSC if "B" in phases else 0):
                xa_buf = load_sc(sc)
                with tc.For_i(0, SCCOLS, CCOLS,
                              hint_engines=(mybir.EngineType.PE,)) as iv:
                    xa_step = spool.tile([128, 2 * G, CCOLS], BF16, tag="xa_step")
                    nc.vector.tensor_copy(out=xa_step[:], in_=xa_buf[:, :, ds(iv, CCOLS)])
                    for s in range(CHUNK):
                        so = s * BSH           # static within-chunk offset
                        # h(t-1) lives in hstage slot so-16 (previous step's
                        # write; step 0 reads the last slot of the previous
                        # chunk, memset to 0 before the first chunk).
                        po = (so - BSH) % CCOLS
                        hprev = [hstage[:, kc, po : po + BSH] for kc in range(2)]

                        # Per-gate PSUM tiles: a relu's dependency covers only
                        # its own gate's matmuls (PSUM deps are tile-granular),
                        # so each layer transition hides under the other
                        # gates' matmul stream.
                        act1 = spool.tile([128, 256], BF16, tag="act1")
                        act2 = spool.tile([128, 256], BF16, tag="act2")
                        afull = spool.tile([128, 128], F32, tag="afull")
                        gact = spool.tile([128, 96], F32, tag="gact")
                        # one PSUM bank per gate: [a1 64 | a2 64 | a3 32]
                        pg = {g: pspool.tile([128, 160], F32, tag=f"pg_{g}", name=f"pg_{g}_{s}")
                              for g in GATE_ORDER}
                        # ---- L1: h[256] -> 2048, per gate ----
                        for i, g in enumerate(GATE_ORDER):
                            a1 = pg[g][:, 0:64]
                            for mg in range(4):
                                for kc in range(2):
                                    nc.tensor.matmul(
                                        out=pg[g][:, mg * 16 : mg * 16 + 16],
                                        lhsT=wh1[:, (g * 2 + kc) * 512 + mg * 128 : (g * 2 + kc) * 512 + (mg + 1) * 128],
                                        rhs=hprev[kc],
                                        start=(kc == 0), stop=(kc == 1),
                                    )
                            sl = slice(g * 64, g * 64 + 64)
                            if i % 2 == 0:
                                nc.vector.tensor_scalar_max(act1[:, sl], a1, 0.0)
                            else:
                                nc.scalar.activation(act1[:, sl], a1, AF.Relu)
                        # ---- L2 / L3 / gate chain, software-pipelined ----
                        # Each gate's L3 + add + sigma/tanh is emitted right
                        # after its relu2, wedged between later gates' L2
                        # blocks, so the t12/cnew/tanh(c) chain runs during
                        # the matmul stream. Only add_O -> sigmoid_O -> h is
                        # exposed after the last matmul.
                        t12 = spool.tile([128, 64], F32, tag="t12")
                        tc_ = spool.tile([128, 32], F32, tag="tc_")

                        def emit_l2(i, g):
                            a2 = pg[g][:, 64:128]
                            for mg in range(4):
                                for kc in range(4):
                                    nc.tensor.matmul(
                                        out=pg[g][:, 64 + mg * 16 : 64 + mg * 16 + 16],
                                        lhsT=wh2[:, (g * 4 + kc) * 512 + mg * 128 : (g * 4 + kc) * 512 + (mg + 1) * 128],
                                        rhs=act1[:, (g * 4 + kc) * 16 : (g * 4 + kc) * 16 + 16],
                                        start=(kc == 0), stop=(kc == 3),
                                    )
                            sl = slice(g * 64, g * 64 + 64)
                            if i % 2 == 1:
                                nc.vector.tensor_scalar_max(act2[:, sl], a2, 0.0)
                            else:
                                nc.scalar.activation(act2[:, sl], a2, AF.Relu)

                        def emit_l3(g):
                            a3 = pg[g][:, 128:160]
                            for j in range(2):
                                for kc in range(4):
                                    nc.tensor.matmul(
                                        out=pg[g][:, 128 + j * 16 : 128 + j * 16 + 16],
                                        lhsT=wh3[:, (g * 4 + kc) * 256 + j * 128 : (g * 4 + kc) * 256 + (j + 1) * 128],
                                        rhs=act2[:, (g * 4 + kc) * 16 : (g * 4 + kc) * 16 + 16],
                                        start=(kc == 0), stop=(kc == 3),
                                    )
                            nc.vector.tensor_add(
                                out=afull[:, g * 32 : g * 32 + 32].rearrange("p (j b) -> p j b", j=2),
                                in0=a3.rearrange("p (j b) -> p j b", j=2),
                                in1=xa_step[:, 2 * g : 2 * g + 2, so : so + BSH],
                            )
                            if g == 3:
                                nc.scalar.activation(z[:, 32:64], afull[:, 96:128],
                                                     AF.Tanh, scale=DESCALE)
                            else:
                                nc.scalar.activation(gact[:, g * 32 : g * 32 + 32],
                                                     afull[:, g * 32 : g * 32 + 32],
                                                     AF.Sigmoid, scale=DESCALE)

                        emit_l2(0, 3)          # Ch
                        emit_l2(1, 0)          # F
                        emit_l2(2, 1)          # I
                        emit_l3(3)             # L3_Ch + add + tanh -> z
                        emit_l2(3, 2)          # O
                        emit_l3(0)             # L3_F + add + sigmoid
                        emit_l3(1)             # L3_I + add + sigmoid
                        # t12 = [F*c | I*tanh(Ch)]; cnew = t12[0:32]+t12[32:64]
                        nc.vector.tensor_mul(t12[:], gact[:, 0:64], z[:])
                        nc.vector.tensor_add(z[:, 0:32], t12[:, 0:32], t12[:, 32:64])
                        nc.scalar.activation(tc_[:], z[:, 0:32], AF.Tanh)
                        emit_l3(2)             # L3_O + add + sigmoid (exposed)
                        # h = O * tanh(c) -> directly into hstage slot
                        nc.vector.tensor_mul(
                            out=hstage[:, :, so : so + BSH],
                            in0=gact[:, 64:96].rearrange("p (j b) -> p j b", j=2),
                            in1=tc_[:].rearrange("p (j b) -> p j b", j=2),
                        )

                    if debug:
                        nc.vector.tensor_copy(
                            out=hsb[:, :, ds(iv + sc * SCCOLS, CCOLS)], in_=hstage[:]
                        )
                    # ---- attention block for this chunk (phase C inlined) ----
                    ez = spool.tile([128, 2, CCOLS], F32, tag="ez")
                    for mc in range(2):
                        z2 = cpspool.tile([128, CCOLS], F32, tag="z2")
                        for kc in range(2):
                            nc.tensor.matmul(
                                out=z2[:],
                                lhsT=wa[:, kc * 256 + mc * 128 : kc * 256 + (mc + 1) * 128],
                                rhs=hstage[:, kc, :],
                                start=(kc == 0), stop=(kc == 1),
                            )
                        nc.scalar.activation(ez[:, mc, :], z2[:], AF.Tanh)
                        nc.scalar.activation(ez[:, mc, :], ez[:, mc, :], AF.Exp)
                    prod = spool.tile([128, 2, CCOLS], F32, tag="prod")
                    nc.gpsimd.tensor_mul(prod[:], ez[:], hstage[:])
                    nc.gpsimd.tensor_add(cacc[:], cacc[:], prod[:])
                    nc.gpsimd.tensor_add(nacc[:], nacc[:], ez[:])

            # ================= Phase C epilogue: reduce + output =================
            bstack.close()
            with tc.tile_pool(name="c_sb", bufs=3) as csb:
                # tree-reduce over the 32 t-local slots (cols = tl*16 + b)
                for half in (16, 8, 4, 2, 1):
                    w = half * BSH
                    nc.vector.tensor_add(cacc[:, :, 0:w], cacc[:, :, 0:w], cacc[:, :, w : 2 * w])
                    nc.vector.tensor_add(nacc[:, :, 0:w], nacc[:, :, 0:w], nacc[:, :, w : 2 * w])
                ctx = csb.tile([128, 2, BSH], F32, tag="ctx")
                rcp = csb.tile([128, 2, BSH], F32, tag="rcp")
                nc.vector.reciprocal(rcp[:], nacc[:, :, 0:BSH])
                nc.vector.tensor_mul(ctx[:], cacc[:, :, 0:BSH], rcp[:])
                nc.sync.dma_start(out=out_d.rearrange("j p b -> p j b"), in_=ctx[:])
                if debug:
                    nc.sync.dma_start(out=hs_dump[:], in_=hsb[:])

    _legalize_waits(nc)
    if _LDW_OPT:
        _patch_walrus_ldw_opt()
        _make_self_loading(nc)
    return nc


def _bf16(a):
    return np.ascontiguousarray(a).astype(ml_dtypes.bfloat16)


def _f8(a):
    return np.ascontiguousarray(a).astype(ml_dtypes.float8_e4m3fn)


def prep_weights(Wh1, Wh2, Wh3, Wx1, Wx2, Wx3, Wa):
    """Host-side: pre-transpose weights into SBUF layouts.
    Layout: [128 rows of din-chunk, g*KC*dout + kc*dout + m].
    h-weights: fp8e4m3 scaled by 64 per layer. Wx3: scaled by 2^18 so xa
    matches the h-path's cumulative 64^3 scale."""
    def wl(W, kc, dout, scale=1.0):
        return np.transpose((W * scale).reshape(G, kc, 128, dout), (2, 0, 1, 3)).reshape(128, G * kc * dout)

    return {
        "wh1": _f8(wl(Wh1, 2, 512, WH_SCALE)),
        "wh2": _f8(wl(Wh2, 4, 512, WH_SCALE)),
        "wh3": _f8(wl(Wh3, 4, 256, WH_SCALE)),
        "wx1": _bf16(wl(Wx1, 1, 512)),
        "wx2": _bf16(wl(Wx2, 4, 512)),
        "wx3": _bf16(wl(Wx3, 4, 256, WH_SCALE ** 3)),
        "wa": _bf16(np.transpose(Wa.reshape(2, 128, 256), (1, 0, 2)).reshape(128, 512)),
    }


def kernel(x, Wh1, bh1, Wh2, bh2, Wh3, bh3, Wx1, bx1, Wx2, bx2, Wx3, bx3, Wa, ba,
           _T=None, _ncores=NCORE, _trace=False):
    from concourse.bass_utils import run_bass_kernel_spmd

    x = np.asarray(x, dtype=np.float32)
    for b_ in (bh1, bh2, bh3, bx1, bx2, bx3, ba):
        assert np.all(np.asarray(b_) == 0.0), "kernel assumes zero biases"

    T = x.shape[1] if _T is None else _T
    nc = build(T)
    wmap = prep_weights(np.asarray(Wh1), np.asarray(Wh2), np.asarray(Wh3),
                        np.asarray(Wx1), np.asarray(Wx2), np.asarray(Wx3),
                        np.asarray(Wa))
    in_maps = []
    for c in range(_ncores):
        xc = x[c * BSH : (c + 1) * BSH, :T]                     # [16, T, 128]
        xTc = _bf16(np.transpose(xc, (2, 1, 0)).reshape(IN, T * BSH))
        m = dict(wmap)
        m["xT"] = xTc
        in_maps.append(m)

    res = run_bass_kernel_spmd(nc, in_maps, list(range(_ncores)),
                               trace=_trace, trace_cores=[0] if _trace else None)
    out = np.empty((B, H), dtype=np.float32)
    for c in range(_ncores):
        o = res.results[c]["out"]                                # [2, 128, 16]
        out[c * BSH : (c + 1) * BSH] = np.transpose(o, (2, 0, 1)).reshape(BSH, H)
    if _trace:
        return out, res
    return out


def golden(x, Wh1, Wh2, Wh3, Wx1, Wx2, Wx3, Wa, T):
    """Plain fp32 numpy reference (for debugging small T)."""
    x = x[:, :T].astype(np.float32)
    Bn = x.shape[0]

    def sig(a):
        return 1.0 / (1.0 + np.exp(-a))

    def dnn4(inp, W1, W2, W3):
        h = np.maximum(np.einsum("bi,gio->gbo", inp, W1), 0)
        h = np.maximum(np.einsum("gbi,gio->gbo", h, W2), 0)
        return np.einsum("gbi,gio->gbo", h, W3)

    h = np.zeros((Bn, H), np.float32)
    c = np.zeros((Bn, H), np.float32)
    hs = np.zeros((T, Bn, H), np.float32)
    for t in range(T):
        a = dnn4(h, Wh1, Wh2, Wh3) + dnn4(x[:, t], Wx1, Wx2, Wx3)
        Fg, Ig, Og, Ch = sig(a[0]), sig(a[1]), sig(a[2]), np.tanh(a[3])
        c = Fg * c + Ig * Ch
        h = Og * np.tanh(c)
        hs[t] = h
    z = np.tanh(np.einsum("tbh,hk->tbk", hs, Wa))
    e = np.exp(z - z.max(axis=0, keepdims=True))
    aw = e / e.sum(axis=0, keepdims=True)
    return (aw * hs).sum(axis=0)


if __name__ == "__main__":
    rng = np.random.default_rng(0)
    s = 0.02
    T = int(sys.argv[1]) if len(sys.argv) > 1 else 64
    inp = {
        "x": rng.standard_normal((B, T_FULL, IN), dtype=np.float32),
        "Wh1": (rng.standard_normal((G, H, M1)) * s).astype(np.float32),
        "bh1": np.zeros((G, M1), np.float32),
        "Wh2": (rng.standard_normal((G, M1, M2)) * s).astype(np.float32),
        "bh2": np.zeros((G, M2), np.float32),
        "Wh3": (rng.standard_normal((G, M2, H)) * s).astype(np.float32),
        "bh3": np.zeros((G, H), np.float32),
        "Wx1": (rng.standard_normal((G, IN, M1)) * s).astype(np.float32),
        "bx1": np.zeros((G, M1), np.float32),
        "Wx2": (rng.standard_normal((G, M1, M2)) * s).astype(np.float32),
        "bx2": np.zeros((G, M2), np.float32),
        "Wx3": (rng.standard_normal((G, M2, H)) * s).astype(np.float32),
        "bx3": np.zeros((G, H), np.float32),
        "Wa": (rng.standard_normal((H, H)) * s).astype(np.float32),
        "ba": np.zeros((H,), np.float32),
    }
    exp = golden(inp["x"], inp["Wh1"], inp["Wh2"], inp["Wh3"],
                 inp["Wx1"], inp["Wx2"], inp["Wx3"], inp["Wa"], T)
    got = kernel(**inp, _T=T)
    err = np.abs(got - exp)
    print("selftest T=%d  absmax err %.3e  rel %.3e"
          % (T, err.max(), err.max() / np.abs(exp).max()))


# revision 15
# speedup vs baseline: 1.0188x; 1.0188x over previous
"""DeepLSTM Trainium2 kernel (nn_DeepLSTM_1365799600435).

Strategy: data-parallel over batch (B=128 -> 16 rows/core on 8 cores, no
collectives). Per core:
  Phase A: x-path 3-layer MLPs (4 gates) precomputed for all T in big
           weight-stationary bf16 matmuls; result xa spilled to DRAM.
           Wx3 is pre-scaled by 2^18 so xa arrives at the same scale as
           the fp8 h-path output (see below).
  Phase B: sequential LSTM recurrence over T=1024 steps. Weight-stationary
           orientation keeps every layer's activations in [feature, batch]
           form so no transposes are needed. h-path weights are fp8e4m3
           scaled by 64 per layer (halves LDWEIGHTS traffic, which binds
           the matmul issue rate at N=16); activations stay bf16 (mixed
           dtype matmuls). The cumulative 64^3 = 2^18 scale is divided
           out for free in the sigmoid/tanh activation `scale` param.
           Gates are computed in order Ch,F,I,O with per-gate relus and
           adds so each layer transition's PE->DVE semaphore latency
           hides under the other gates' matmuls. Dummy matmuls fill the
           cell-update tail to keep the PE HAM clock-gate at 2.4 GHz.
  Phase C: attention over T. exp without max-subtraction (logits are
           tanh-bounded), strided accumulation, fp32 accumulators.

All dynamic addressing uses register-offset APs on compute instructions
(this toolchain rejects register-offset DMA), with xa staged per
super-chunk through static DMAs.
"""

import os
import sys

import numpy as np
import ml_dtypes

for _p in ("/opt/trn_rl_repo", "/root/.axon_site/_ro/trn_rl_repo"):
    if os.path.isdir(_p) and _p not in sys.path:
        sys.path.append(_p)

import concourse.bass as bass
import concourse.mybir as mybir
import concourse.tile as tile
from concourse.bass import ds

F32 = mybir.dt.float32
BF16 = mybir.dt.bfloat16
F8 = mybir.dt.float8e4
WH_SCALE = 64.0  # h-weights stored as fp8e4m3 scaled by 64 (per layer)
DESCALE = 1.0 / (WH_SCALE ** 3)  # folded into the gate activation scale
AF = mybir.ActivationFunctionType

# Problem constants
B, T_FULL, IN, H = 128, 1024, 128, 256
M1 = M2 = 512
G = 4
NCORE = 8
BSH = B // NCORE  # 16 batch rows per core

CHUNK = 32          # recurrence steps per For_i iteration
CCOLS = CHUNK * BSH  # cols per chunk

N_DUMMY = int(os.environ.get("KERNEL_NDUMMY", "0"))
DUMMY_N = int(os.environ.get("KERNEL_DUMMYN", "384"))

_LDW_OPT = os.environ.get("KERNEL_LDW_OPT", "1") == "1"
_ldw_patched = [False]


def _patch_walrus_ldw_opt():
    if _ldw_patched[0] or not _LDW_OPT:
        return
    import concourse.bass_utils as _bu
    _orig = _bu.run_command

    def _patched(argv, **kw):
        argv = ["--enable-ldw-opt=true" if a == "--enable-ldw-opt=false" else a
                for a in argv]
        return _orig(argv, **kw)

    _bu.run_command = _patched
    _ldw_patched[0] = True


def _make_self_loading(nc):
    """Fold standalone InstLdweights into their matmuls (required by
    walrus --enable-ldw-opt=true, which overlaps weight loads with the
    previous matmul via the background weight buffer)."""
    n_conv = 0
    for func in nc.m.functions:
        for block in func.blocks:
            insts = block.instructions
            keep = []
            for inst in insts:
                cls = type(inst).__name__
                if cls == "InstLdweights":
                    n_conv += 1
                    if inst.sync_info and (inst.sync_info.on_wait or inst.sync_info.on_update):
                        nop = mybir.InstNoOp(name=nc.get_next_instruction_name(),
                                             engine=inst.engine, sync_info=inst.sync_info,
                                             bass_nofuse=True)
                        keep.append(nop)
                    continue
                if cls == "InstMatmult":
                    inst.ldweights = True
                keep.append(inst)
            insts[:] = keep
    return n_conv


def _legalize_waits(nc):
    """This walrus build accepts at most 1 sem-wait per instruction (2 for
    EventSemaphore ops). Tile sometimes attaches more (final drain, loop
    reset blocks): hoist extras onto same-engine NoOps inserted before."""
    n_split = 0
    for func in nc.m.functions:
        for block in func.blocks:
            insts = block.instructions
            i = 0
            while i < len(insts):
                inst = insts[i]
                si = inst.sync_info
                if si is None or not si.on_wait:
                    i += 1
                    continue
                cap = 2 if "EventSemaphore" in type(inst).__name__ else 1
                waits = list(si.on_wait)
                if len(waits) <= cap:
                    i += 1
                    continue
                keep, hoist = waits[-cap:], waits[:-cap]
                carriers = [
                    mybir.InstNoOp(
                        name=nc.get_next_instruction_name(),
                        engine=inst.engine,
                        sync_info=mybir.SyncInfo(on_wait=[w], on_update=[]),
                        bass_nofuse=True,
                    )
                    for w in hoist
                ]
                inst.sync_info = mybir.SyncInfo(on_wait=keep, on_update=list(si.on_update))
                insts[i:i] = carriers
                n_split += 1
                i += 1 + len(carriers)
    return n_split


# Gate order for phase B emission: Ch first so the t12/cnew/tanh chain can
# overlap the later gates' matmuls; O last so only add_O -> sigmoid -> h is
# exposed after the final matmul burst.
# Gate indices in the weight layout: F=0, I=1, O=2, Ch=3.
GATE_ORDER = (3, 0, 1, 2)


def build(T=T_FULL, sc_chunks=8, debug=False, phases="ABC"):
    """Build the per-core Bass program. T must be a multiple of 32."""
    assert T % 32 == 0
    COLS = T * BSH
    NCHUNK = COLS // CCOLS              # recurrence chunks
    sc_chunks = min(sc_chunks, NCHUNK)
    assert NCHUNK % sc_chunks == 0
    NSC = NCHUNK // sc_chunks           # super-chunks
    SCCOLS = sc_chunks * CCOLS          # cols per super-chunk
    NBLK = COLS // 512                  # 512-col blocks for phases A and C

    nc = bass.Bass()

    # ---- DRAM I/O (host pre-arranges layouts; see kernel()) ----
    xT_d = nc.dram_tensor("xT", [IN, COLS], BF16, kind="ExternalInput")
    wx1_d = nc.dram_tensor("wx1", [128, G * 512], BF16, kind="ExternalInput")
    wx2_d = nc.dram_tensor("wx2", [128, G * 4 * 512], BF16, kind="ExternalInput")
    wx3_d = nc.dram_tensor("wx3", [128, G * 4 * 256], BF16, kind="ExternalInput")
    wh1_d = nc.dram_tensor("wh1", [128, G * 2 * 512], F8, kind="ExternalInput")
    wh2_d = nc.dram_tensor("wh2", [128, G * 4 * 512], F8, kind="ExternalInput")
    wh3_d = nc.dram_tensor("wh3", [128, G * 4 * 256], F8, kind="ExternalInput")
    wa_d = nc.dram_tensor("wa", [128, 2 * 256], BF16, kind="ExternalInput")
    out_d = nc.dram_tensor("out", [2, 128, BSH], F32, kind="ExternalOutput")

    # xa spill: [gm, p, col]; gm = g*2 + j (j = output h-chunk), col = t*16+b
    xa_d = nc.dram_tensor("xa_d", [2 * G, 128, COLS], BF16,
                          kind="ExternalOutput" if debug else "Internal")
    hs_dump = nc.dram_tensor("hs_dump", [128, 2, COLS], BF16,
                             kind="ExternalOutput") if debug else None

    # ================= Phase A: x-path MLPs =================
    if "A" in phases:
      with tile.TileContext(nc) as tc:
          with (
              tc.tile_pool(name="a_w", bufs=1) as wpool,
              tc.tile_pool(name="a_ps", bufs=8, space="PSUM") as pspool,
              tc.tile_pool(name="a_sb", bufs=3) as spool,
          ):
              xT = wpool.tile([128, COLS], BF16)
              wx1 = wpool.tile([128, G * 512], BF16)
              wx2 = wpool.tile([128, G * 4 * 512], BF16)
              wx3 = wpool.tile([128, G * 4 * 256], BF16)
              nc.sync.dma_start(out=xT[:], in_=xT_d[:])
              nc.sync.dma_start(out=wx1[:], in_=wx1_d[:])
              nc.sync.dma_start(out=wx2[:], in_=wx2_d[:])
              nc.sync.dma_start(out=wx3[:], in_=wx3_d[:])

              for blk in range(NBLK):
                  c0 = blk * 512
                  for g in range(G):
                      # L1: [128 in] -> 512, K=1 chunk
                      ps1 = [pspool.tile([128, 512], F32, tag="ps", name=f"ps1_{blk}_{g}_{i}") for i in range(4)]
                      for mc in range(4):
                          nc.tensor.matmul(
                              out=ps1[mc][:],
                              lhsT=wx1[:, g * 512 + mc * 128 : g * 512 + (mc + 1) * 128],
                              rhs=xT[:, c0 : c0 + 512],
                              start=True, stop=True,
                          )
                      act1 = spool.tile([128, 4, 512], BF16, tag="act1")
                      for mc in range(4):
                          nc.vector.tensor_scalar_max(act1[:, mc, :], ps1[mc][:], 0.0)
                      # L2: 512 -> 512, K=4 chunks
                      ps2 = [pspool.tile([128, 512], F32, tag="ps", name=f"ps2_{blk}_{g}_{i}") for i in range(4)]
                      for mc in range(4):
                          for kc in range(4):
                              nc.tensor.matmul(
                                  out=ps2[mc][:],
                                  lhsT=wx2[:, (g * 4 + kc) * 512 + mc * 128 : (g * 4 + kc) * 512 + (mc + 1) * 128],
                                  rhs=act1[:, kc, :],
                                  start=(kc == 0), stop=(kc == 3),
                              )
                      act2 = spool.tile([128, 4, 512], BF16, tag="act2")
                      for mc in range(4):
                          nc.scalar.activation(act2[:, mc, :], ps2[mc][:], AF.Relu)
                      # L3: 512 -> 256, K=4 chunks
                      ps3 = [pspool.tile([128, 512], F32, tag="ps", name=f"ps3_{blk}_{g}_{i}") for i in range(2)]
                      for mc in range(2):
                          for kc in range(4):
                              nc.tensor.matmul(
                                  out=ps3[mc][:],
                                  lhsT=wx3[:, (g * 4 + kc) * 256 + mc * 128 : (g * 4 + kc) * 256 + (mc + 1) * 128],
                                  rhs=act2[:, kc, :],
                                  start=(kc == 0), stop=(kc == 3),
                              )
                      xa_sb = spool.tile([128, 2, 512], BF16, tag="xa_sb")
                      for mc in range(2):
                          nc.vector.tensor_copy(xa_sb[:, mc, :], ps3[mc][:])
                      nc.sync.dma_start(
                          out=xa_d[2 * g : 2 * g + 2, :, c0 : c0 + 512].rearrange("j p c -> p j c"),
                          in_=xa_sb[:],
                      )

    # ================= Phases B + C =================
    with tile.TileContext(nc) as tc:
        from contextlib import ExitStack
        with (
            tc.tile_pool(name="b_w", bufs=1) as wpool,
            tc.tile_pool(name="b_state", bufs=1) as stpool,
        ):
            bstack = ExitStack()
            pspool = bstack.enter_context(tc.tile_pool(name="b_ps", bufs=1, space="PSUM"))
            spool = bstack.enter_context(tc.tile_pool(name="b_sb", bufs=2))
            wh1 = wpool.tile([128, G * 2 * 512], F8)
            wh2 = wpool.tile([128, G * 4 * 512], F8)
            wh3 = wpool.tile([128, G * 4 * 256], F8)
            wa = wpool.tile([128, 2 * 256], BF16)
            nc.sync.dma_start(out=wh1[:], in_=wh1_d[:])
            nc.sync.dma_start(out=wh2[:], in_=wh2_d[:])
            nc.sync.dma_start(out=wh3[:], in_=wh3_d[:])
            nc.sync.dma_start(out=wa[:], in_=wa_d[:])

            # hs history (bf16): col t*16+b = hn(t), per h-chunk j
            hsb = stpool.tile([128, 2, COLS], BF16)
            if "B" not in phases:
                nc.vector.memset(hsb[:], 0.0)
            # z: [c | tanh(Ch)] -- z[:, 0:32] is the persistent cell state
            z = stpool.tile([128, 64], F32)
            hstage = stpool.tile([128, 2, CCOLS], BF16)  # chunk history staging
            nc.vector.memset(z[:], 0.0)
            nc.vector.memset(hstage[:], 0.0)

            # xa staging: [p, gm, SCCOLS] per super-chunk (single buffer)
            xa_bufs = [stpool.tile([128, 2 * G, SCCOLS], BF16, name="xab0")]

            def load_sc(sc):
                buf = xa_bufs[sc % len(xa_bufs)]
                nc.sync.dma_start(
                    out=buf[:],
                    in_=xa_d[:, :, sc * SCCOLS : (sc + 1) * SCCOLS].rearrange("g p c -> p g c"),
                )
                return buf

            for sc in range(NSC if "B" in phases else 0):
                xa_buf = load_sc(sc)
                with tc.For_i(0, SCCOLS, CCOLS,
                              hint_engines=(mybir.EngineType.PE,)) as iv:
                    xa_step = spool.tile([128, 2 * G, CCOLS], BF16, tag="xa_step")
                    nc.vector.tensor_copy(out=xa_step[:], in_=xa_buf[:, :, ds(iv, CCOLS)])
                    for s in range(CHUNK):
                        so = s * BSH           # static within-chunk offset
                        # h(t-1) lives in hstage slot so-16 (previous step's
                        # write; step 0 reads the last slot of the previous
                        # chunk, memset to 0 before the first chunk).
                        po = (so - BSH) % CCOLS
                        hprev = [hstage[:, kc, po : po + BSH] for kc in range(2)]

                        # Per-gate PSUM tiles: a relu's dependency covers only
                        # its own gate's matmuls (PSUM deps are tile-granular),
                        # so each layer transition hides under the other
                        # gates' matmul stream.
                        act1 = spool.tile([128, 256], BF16, tag="act1")
                        act2 = spool.tile([128, 256], BF16, tag="act2")
                        afull = spool.tile([128, 128], F32, tag="afull")
                        gact = spool.tile([128, 96], F32, tag="gact")
                        # one PSUM bank per gate: [a1 64 | a2 64 | a3 32]
                        pg = {g: pspool.tile([128, 160], F32, tag=f"pg_{g}", name=f"pg_{g}_{s}")
                              for g in GATE_ORDER}
                        # ---- L1: h[256] -> 2048, per gate ----
                        for i, g in enumerate(GATE_ORDER):
                            a1 = pg[g][:, 0:64]
                            for mg in range(4):
                                for kc in range(2):
                                    nc.tensor.matmul(
                                        out=pg[g][:, mg * 16 : mg * 16 + 16],
                                        lhsT=wh1[:, (g * 2 + kc) * 512 + mg * 128 : (g * 2 + kc) * 512 + (mg + 1) * 128],
                                        rhs=hprev[kc],
                                        start=(kc == 0), stop=(kc == 1),
                                    )
                            sl = slice(g * 64, g * 64 + 64)
                            if i % 2 == 0:
                                nc.vector.tensor_scalar_max(act1[:, sl], a1, 0.0)
                            else:
                                nc.scalar.activation(act1[:, sl], a1, AF.Relu)
                        # ---- L2 / L3 / gate chain, software-pipelined ----
                        # Each gate's L3 + add + sigma/tanh is emitted right
                        # after its relu2, wedged between later gates' L2
                        # blocks, so the t12/cnew/tanh(c) chain runs during
                        # the matmul stream. Only add_O -> sigmoid_O -> h is
                        # exposed after the last matmul.
                        t12 = spool.tile([128, 64], F32, tag="t12")
                        tc_ = spool.tile([128, 32], F32, tag="tc_")

                        def emit_l2(i, g):
                            a2 = pg[g][:, 64:128]
                            for mg in range(4):
                                for kc in range(4):
                                    nc.tensor.matmul(
                                        out=pg[g][:, 64 + mg * 16 : 64 + mg * 16 + 16],
                                        lhsT=wh2[:, (g * 4 + kc) * 512 + mg * 128 : (g * 4 + kc) * 512 + (mg + 1) * 128],
                                        rhs=act1[:, (g * 4 + kc) * 16 : (g * 4 + kc) * 16 + 16],
                                        start=(kc == 0), stop=(kc == 3),
                                    )
                            sl = slice(g * 64, g * 64 + 64)
                            if i % 2 == 1:
                                nc.vector.tensor_scalar_max(act2[:, sl], a2, 0.0)
                            else:
                                nc.scalar.activation(act2[:, sl], a2, AF.Relu)

                        def emit_l3(g):
                            a3 = pg[g][:, 128:160]
                            for j in range(2):
                                for kc in range(4):
                                    nc.tensor.matmul(
                                        out=pg[g][:, 128 + j * 16 : 128 + j * 16 + 16],
                                        lhsT=wh3[:, (g * 4 + kc) * 256 + j * 128 : (g * 4 + kc) * 256 + (j + 1) * 128],
                                        rhs=act2[:, (g * 4 + kc) * 16 : (g * 4 + kc) * 16 + 16],
                                        start=(kc == 0), stop=(kc == 3),
                                    )
                            nc.vector.tensor_add(
                                out=afull[:, g * 32 : g * 32 + 32].rearrange("p (j b) -> p j b", j=2),
                                in0=a3.rearrange("p (j b) -> p j b", j=2),
                                in1=xa_step[:, 2 * g : 2 * g + 2, so : so + BSH],
                            )
                            if g == 3:
                                nc.scalar.activation(z[:, 32:64], afull[:, 96:128],
                                                     AF.Tanh, scale=DESCALE)
                            else:
                                nc.scalar.activation(gact[:, g * 32 : g * 32 + 32],
                                                     afull[:, g * 32 : g * 32 + 32],
                                                     AF.Sigmoid, scale=DESCALE)

                        emit_l2(0, 3)          # Ch
                        emit_l2(1, 0)          # F
                        emit_l2(2, 1)          # I
                        emit_l3(3)             # L3_Ch + add + tanh -> z
                        emit_l2(3, 2)          # O
                        emit_l3(0)             # L3_F + add + sigmoid
                        emit_l3(1)             # L3_I + add + sigmoid
                        # t12 = [F*c | I*tanh(Ch)]; cnew = t12[0:32]+t12[32:64]
                        nc.vector.tensor_mul(t12[:], gact[:, 0:64], z[:])
                        nc.vector.tensor_add(z[:, 0:32], t12[:, 0:32], t12[:, 32:64])
                        nc.scalar.activation(tc_[:], z[:, 0:32], AF.Tanh)
                        emit_l3(2)             # L3_O + add + sigmoid (exposed)
                        # h = O * tanh(c) -> directly into hstage slot
                        nc.vector.tensor_mul(
                            out=hstage[:, :, so : so + BSH],
                            in0=gact[:, 64:96].rearrange("p (j b) -> p j b", j=2),
                            in1=tc_[:].rearrange("p (j b) -> p j b", j=2),
                        )

                    nc.vector.tensor_copy(
                        out=hsb[:, :, ds(iv + sc * SCCOLS, CCOLS)], in_=hstage[:]
                    )

            # ================= Phase C: attention =================
            bstack.close()
            do_c = "C" in phases
            cacc = stpool.tile([128, 2, 512], F32)
            nacc = stpool.tile([128, 2, 512], F32)
            nc.vector.memset(cacc[:], 0.0)
            nc.vector.memset(nacc[:], 0.0)
            with tc.tile_pool(name="c_ps", bufs=4, space="PSUM") as cps, \
                 tc.tile_pool(name="c_sb", bufs=3) as csb:
                for blk in range(NBLK if do_c else 0):
                    c0 = blk * 512
                    ez = csb.tile([128, 2, 512], F32, tag="ez")
                    for mc in range(2):
                        z2 = cps.tile([128, 512], F32, tag="z")
                        for kc in range(2):
                            nc.tensor.matmul(
                                out=z2[:],
                                lhsT=wa[:, kc * 256 + mc * 128 : kc * 256 + (mc + 1) * 128],
                                rhs=hsb[:, kc, c0 : c0 + 512],
                                start=(kc == 0), stop=(kc == 1),
                            )
                        nc.scalar.activation(ez[:, mc, :], z2[:], AF.Tanh)
                        nc.scalar.activation(ez[:, mc, :], ez[:, mc, :], AF.Exp)
                    prod = csb.tile([128, 2, 512], F32, tag="prod")
                    nc.vector.tensor_mul(prod[:], ez[:], hsb[:, :, c0 : c0 + 512])
                    nc.vector.tensor_add(cacc[:], cacc[:], prod[:])
                    nc.vector.tensor_add(nacc[:], nacc[:], ez[:])
                # tree-reduce over the 32 t-local slots (cols = tl*16 + b)
                for half in (16, 8, 4, 2, 1):
                    w = half * BSH
                    nc.vector.tensor_add(cacc[:, :, 0:w], cacc[:, :, 0:w], cacc[:, :, w : 2 * w])
                    nc.vector.tensor_add(nacc[:, :, 0:w], nacc[:, :, 0:w], nacc[:, :, w : 2 * w])
                ctx = csb.tile([128, 2, BSH], F32, tag="ctx")
                rcp = csb.tile([128, 2, BSH], F32, tag="rcp")
                nc.vector.reciprocal(rcp[:], nacc[:, :, 0:BSH])
                nc.vector.tensor_mul(ctx[:], cacc[:, :, 0:BSH], rcp[:])
                nc.sync.dma_start(out=out_d.rearrange("j p b -> p j b"), in_=ctx[:])
                if debug:
                    nc.sync.dma_start(out=hs_dump[:], in_=hsb[:])

    _legalize_waits(nc)
    if _LDW_OPT:
        _patch_walrus_ldw_opt()
        _make_self_loading(nc)
    return nc


def _bf16(a):
    return np.ascontiguousarray(a).astype(ml_dtypes.bfloat16)


def _f8(a):
    return np.ascontiguousarray(a).astype(ml_dtypes.float8_e4m3fn)


def prep_weights(Wh1, Wh2, Wh3, Wx1, Wx2, Wx3, Wa):
    """Host-side: pre-transpose weights into SBUF layouts.
    Layout: [128 rows of din-chunk, g*KC*dout + kc*dout + m].
    h-weights: fp8e4m3 scaled by 64 per layer. Wx3: scaled by 2^18 so xa
    matches the h-path's cumulative 64^3 scale."""
    def wl(W, kc, dout, scale=1.0):
        return np.transpose((W * scale).reshape(G, kc, 128, dout), (2, 0, 1, 3)).reshape(128, G * kc * dout)

    return {
        "wh1": _f8(wl(Wh1, 2, 512, WH_SCALE)),
        "wh2": _f8(wl(Wh2, 4, 512, WH_SCALE)),
        "wh3": _f8(wl(Wh3, 4, 256, WH_SCALE)),
        "wx1": _bf16(wl(Wx1, 1, 512)),
        "wx2": _bf16(wl(Wx2, 4, 512)),
        "wx3": _bf16(wl(Wx3, 4, 256, WH_SCALE ** 3)),
        "wa": _bf16(np.transpose(Wa.reshape(2, 128, 256), (1, 0, 2)).reshape(128, 512)),
    }


def kernel(x, Wh1, bh1, Wh2, bh2, Wh3, bh3, Wx1, bx1, Wx2, bx2, Wx3, bx3, Wa, ba,
           _T=None, _ncores=NCORE, _trace=False):
    from concourse.bass_utils import run_bass_kernel_spmd

    x = np.asarray(x, dtype=np.float32)
    for b_ in (bh1, bh2, bh3, bx1, bx2, bx3, ba):
        assert np.all(np.asarray(b_) == 0.0), "kernel assumes zero biases"

    T = x.shape[1] if _T is None else _T
    nc = build(T)
    wmap = prep_weights(np.asarray(Wh1), np.asarray(Wh2), np.asarray(Wh3),
                        np.asarray(Wx1), np.asarray(Wx2), np.asarray(Wx3),
                        np.asarray(Wa))
    in_maps = []
    for c in range(_ncores):
        xc = x[c * BSH : (c + 1) * BSH, :T]                     # [16, T, 128]
        xTc = _bf16(np.transpose(xc, (2, 1, 0)).reshape(IN, T * BSH))
        m = dict(wmap)
        m["xT"] = xTc
        in_maps.append(m)

    res = run_bass_kernel_spmd(nc, in_maps, list(range(_ncores)),
                               trace=_trace, trace_cores=[0] if _trace else None)
    out = np.empty((B, H), dtype=np.float32)
    for c in range(_ncores):
        o = res.results[c]["out"]                                # [2, 128, 16]
        out[c * BSH : (c + 1) * BSH] = np.transpose(o, (2, 0, 1)).reshape(BSH, H)
    if _trace:
        return out, res
    return out


def golden(x, Wh1, Wh2, Wh3, Wx1, Wx2, Wx3, Wa, T):
    """Plain fp32 numpy reference (for debugging small T)."""
    x = x[:, :T].astype(np.float32)
    Bn = x.shape[0]

    def sig(a):
        return 1.0 / (1.0 + np.exp(-a))

    def dnn4(inp, W1, W2, W3):
        h = np.maximum(np.einsum("bi,gio->gbo", inp, W1), 0)
        h = np.maximum(np.einsum("gbi,gio->gbo", h, W2), 0)
        return np.einsum("gbi,gio->gbo", h, W3)

    h = np.zeros((Bn, H), np.float32)
    c = np.zeros((Bn, H), np.float32)
    hs = np.zeros((T, Bn, H), np.float32)
    for t in range(T):
        a = dnn4(h, Wh1, Wh2, Wh3) + dnn4(x[:, t], Wx1, Wx2, Wx3)
        Fg, Ig, Og, Ch = sig(a[0]), sig(a[1]), sig(a[2]), np.tanh(a[3])
        c = Fg * c + Ig * Ch
        h = Og * np.tanh(c)
        hs[t] = h
    z = np.tanh(np.einsum("tbh,hk->tbk", hs, Wa))
    e = np.exp(z - z.max(axis=0, keepdims=True))
    aw = e / e.sum(axis=0, keepdims=True)
    return (aw * hs).sum(axis=0)


if __name__ == "__main__":
    rng = np.random.default_rng(0)
    s = 0.02
    T = int(sys.argv[1]) if len(sys.argv) > 1 else 64
    inp = {
        "x": rng.standard_normal((B, T_FULL, IN), dtype=np.float32),
        "Wh1": (rng.standard_normal((G, H, M1)) * s).astype(np.float32),
        "bh1": np.zeros((G, M1), np.float32),
        "Wh2": (rng.standard_normal((G, M1, M2)) * s).astype(np.float32),
        "bh2": np.zeros((G, M2), np.float32),
        "Wh3": (rng.standard_normal((G, M2, H)) * s).astype(np.float32),
        "bh3": np.zeros((G, H), np.float32),
        "Wx1": (rng.standard_normal((G, IN, M1)) * s).astype(np.float32),
        "bx1": np.zeros((G, M1), np.float32),
        "Wx2": (rng.standard_normal((G, M1, M2)) * s).astype(np.float32),
        "bx2": np.zeros((G, M2), np.float32),
        "Wx3": (rng.standard_normal((G, M2, H)) * s).astype(np.float32),
        "bx3": np.zeros((G, H), np.float32),
        "Wa": (rng.standard_normal((H, H)) * s).astype(np.float32),
        "ba": np.zeros((H,), np.float32),
    }
    exp = golden(inp["x"], inp["Wh1"], inp["Wh2"], inp["Wh3"],
                 inp["Wx1"], inp["Wx2"], inp["Wx3"], inp["Wa"], T)
    got = kernel(**inp, _T=T)
    err = np.abs(got - exp)
    print("selftest T=%d  absmax err %.3e  rel %.3e"
          % (T, err.max(), err.max() / np.abs(exp).max()))
